# revision 6
# baseline (speedup 1.0000x reference)
"""Trainium2 Bass kernel for nn_ATTNLoss (top-k masked attention reconstruction loss).

Math: loss = mean((x-y)^2) + ALPHA * mean((attn - topk32(attn))^2)
Since topk scattering only zeroes the top-32 entries of each row:
    attn_loss = (sum(attn^2) - sum_{rows} sum(top32(row)^2)) / N^2
so nothing sparse needs materializing; only three scalar sums are needed.

Sharding: rows split evenly across 8 NeuronCores (top-k is row-local).
Each core computes per-partition partial sums [128, 3]; the host combines
them in float64 and forms the final scalar.

Per-row top-32 on device: per-row top-8 of each column block (nc.vector.max)
produces a candidate set; 4 rounds of max+match_replace on the narrow
candidate buffer yield the exact top-32 values provided no block holds >8 of
the row's top-32 elements. kernel() verifies that property on the actual
input on the host (cheap numpy check) and falls back to a smaller block size
or a full-width exact variant if ever violated.
"""

import numpy as np

N = 8192  # attention matrix is [N, N]
D = 1024  # reconstruction feature dim
K = 32  # top-k
ALPHA = 0.1
N_CORES = 8
ROWS = N // N_CORES  # rows per core = 1024
P = 128  # SBUF partitions
NT = ROWS // P  # row-tiles per core = 8

_BUILDS: dict = {}


def _build_bass(blk: int):
    """Build the per-core Bass module.

    blk > 0: level-1 block size for the block-top8 candidate pass.
    blk == 0: exact full-width fallback (4 rounds of max+match_replace over
    the whole 8192-wide row).
    """
    import concourse.tile as tile
    from concourse import bacc, mybir

    f32 = mybir.dt.float32
    Sq = mybir.ActivationFunctionType.Square
    AX = mybir.AxisListType.X
    ADD = mybir.AluOpType.add

    # Bacc (not raw Bass): its compile() pass splits multi-wait sync_infos,
    # which the TRN2 ISA requires (at most one wait per instruction).
    nc = bacc.Bacc()
    attn_in = nc.declare_dram_parameter("attn", [ROWS, N], f32, isOutput=False)
    # x and y interleaved as [ROWS, 2*D] (x in cols [0,D), y in [D,2D)) so a
    # single DMA feeds the subtract — keeps per-instruction sem waits low.
    xy_in = nc.declare_dram_parameter("xy", [ROWS, 2 * D], f32, isOutput=False)
    out_ext = nc.declare_dram_parameter("out", [P, 4], f32, isOutput=True)

    with tile.TileContext(nc) as tc:
        with (
            tc.tile_pool(name="attn_p", bufs=3) as attn_p,
            tc.tile_pool(name="xy_p", bufs=2) as xy_p,
            tc.tile_pool(name="small_p", bufs=2) as small_p,
            tc.tile_pool(name="acc_p", bufs=1) as acc_p,
        ):
            # acc columns: [0:NT) sum(x-y)^2, [NT:2NT) sum(attn^2) pieces,
            # [2NT:3NT) sum(top32^2) pieces, [3NT:4NT) fallback extras.
            acc = acc_p.tile([P, 4 * NT], f32)
            nc.vector.memset(acc[:], 0.0)

            for t in range(NT):
                a = attn_p.tile([P, N], f32, tag="a")
                nc.sync.dma_start(out=a[:], in_=attn_in[t * P : (t + 1) * P, :])

                top = small_p.tile([P, K], f32, tag="top")
                if blk > 0:
                    nb = N // blk
                    cw = nb * 8
                    cand = small_p.tile([P, cw], f32, tag="cand")
                    for b in range(nb):
                        nc.vector.max(
                            out=cand[:, b * 8 : (b + 1) * 8],
                            in_=a[:, b * blk : (b + 1) * blk],
                        )
                    for r in range(K // 8):
                        nc.vector.max(out=top[:, r * 8 : (r + 1) * 8], in_=cand[:])
                        if r < K // 8 - 1:
                            nc.vector.match_replace(
                                out=cand[:],
                                in_to_replace=top[:, r * 8 : (r + 1) * 8],
                                in_values=cand[:],
                                imm_value=0.0,
                            )
                    # sum(top32^2) for this tile
                    nc.scalar.activation(
                        out=top[:], in_=top[:], func=Sq,
                        accum_out=acc[:, 2 * NT + t : 2 * NT + t + 1],
                    )
                    # sum(attn^2) for this tile (in-place square; `a` is dead after)
                    nc.scalar.activation(
                        out=a[:], in_=a[:], func=Sq,
                        accum_out=acc[:, NT + t : NT + t + 1],
                    )
                else:
                    # Exact fallback: extract top-32 directly from the full row.
                    # match_replace zeroes the extracted values in `a`, so
                    # sum(attn^2) = sum(a_modified^2) + sum(top24_extracted^2).
                    for r in range(K // 8):
                        nc.vector.max(out=top[:, r * 8 : (r + 1) * 8], in_=a[:])
                        if r < K // 8 - 1:
                            nc.vector.match_replace(
                                out=a[:],
                                in_to_replace=top[:, r * 8 : (r + 1) * 8],
                                in_values=a[:],
                                imm_value=0.0,
                            )
                    # first 24 values were zeroed out of `a`
                    nc.scalar.activation(
                        out=top[:, : K - 8], in_=top[:, : K - 8], func=Sq,
                        accum_out=acc[:, 3 * NT + t : 3 * NT + t + 1],
                    )
                    # last 8 remain in `a`
                    nc.scalar.activation(
                        out=top[:, K - 8 :], in_=top[:, K - 8 :], func=Sq,
                        accum_out=acc[:, 2 * NT + t : 2 * NT + t + 1],
                    )
                    nc.scalar.activation(
                        out=a[:], in_=a[:], func=Sq,
                        accum_out=acc[:, NT + t : NT + t + 1],
                    )

                xyt = xy_p.tile([P, 2 * D], f32, tag="xyt")
                nc.sync.dma_start(out=xyt[:], in_=xy_in[t * P : (t + 1) * P, :])
                nc.vector.tensor_sub(
                    out=xyt[:, 0:D], in0=xyt[:, 0:D], in1=xyt[:, D : 2 * D]
                )
                nc.scalar.activation(
                    out=xyt[:, 0:D], in_=xyt[:, 0:D], func=Sq,
                    accum_out=acc[:, t : t + 1],
                )

            osb = acc_p.tile([P, 4], f32)
            nc.vector.tensor_reduce(out=osb[:, 0:1], in_=acc[:, 0:NT], axis=AX, op=ADD)
            nc.vector.tensor_reduce(
                out=osb[:, 1:2], in_=acc[:, NT : 2 * NT], axis=AX, op=ADD
            )
            # col2: sum(top32^2) = tail-8 partials (+ top-24 partials in fallback)
            nc.vector.tensor_reduce(
                out=osb[:, 2:3], in_=acc[:, 2 * NT : 3 * NT], axis=AX, op=ADD
            )
            nc.vector.tensor_reduce(
                out=osb[:, 3:4], in_=acc[:, 3 * NT : 4 * NT], axis=AX, op=ADD
            )
            nc.sync.dma_start(out=out_ext[:], in_=osb[:])

    nc.finalize()  # runs Bacc.compile(): wait splitting + register allocation
    return nc


def _get_nc(blk: int):
    if blk not in _BUILDS:
        _BUILDS[blk] = _build_bass(blk)
    return _BUILDS[blk]


def _pick_blk(attn: np.ndarray) -> int:
    """Choose the largest safe level-1 block size for this input.

    Safe means: for every row, no block contains more than 8 elements that
    are >= the row's 32nd-largest value (so block-top8 candidates provably
    contain every valid top-32 choice).
    """
    t32 = np.partition(attn, N - K, axis=1)[:, N - K]
    ge = attn >= t32[:, None]
    for blk in (256, 128):
        nb = N // blk
        cnt = ge.reshape(N, nb, blk).sum(axis=2, dtype=np.int32)
        if cnt.max() <= 8:
            return blk
    return 0


def _combine(results) -> np.float32:
    S = np.zeros(4, dtype=np.float64)
    for r in results:
        S += r["out"].astype(np.float64).sum(axis=0)
    sxy, sattn, stop_tail, stop_head = S
    # fallback path splits top32^2 across cols 2 and 3 and leaves the
    # head-24 part out of sattn (match_replace zeroed those entries)
    sattn = sattn + stop_head
    stop = stop_tail + stop_head
    loss = sxy / (N * D) + ALPHA * (sattn - stop) / (N * N)
    return np.float32(loss)


def _shard(x: np.ndarray, y: np.ndarray, attn: np.ndarray):
    in_maps = []
    for c in range(N_CORES):
        r0, r1 = c * ROWS, (c + 1) * ROWS
        in_maps.append(
            {
                "attn": np.ascontiguousarray(attn[r0:r1]),
                "xy": np.concatenate([x[r0:r1], y[r0:r1]], axis=1),
            }
        )
    return in_maps


def kernel(x: np.ndarray, y: np.ndarray, attn: np.ndarray) -> np.ndarray:
    from concourse.bass_utils import run_bass_kernel_spmd

    x = np.asarray(x, dtype=np.float32)
    y = np.asarray(y, dtype=np.float32)
    attn = np.asarray(attn, dtype=np.float32)

    nc = _get_nc(_pick_blk(attn))
    res = run_bass_kernel_spmd(nc, _shard(x, y, attn), list(range(N_CORES)))
    return np.asarray(_combine(res.results))


# revision 10
# speedup vs baseline: 1.2369x; 1.2369x over previous
"""Trainium2 Bass kernel for nn_ATTNLoss (top-k masked attention reconstruction loss).

Math: loss = mean((x-y)^2) + ALPHA * mean((attn - topk32(attn))^2)
Since topk scattering only zeroes the top-32 entries of each row:
    attn_loss = (sum(attn^2) - sum_{rows} sum(top32(row)^2)) / N^2
so nothing sparse needs materializing; only three scalar sums are needed.

Sharding: rows split evenly across 8 NeuronCores (top-k is row-local).
Each core computes per-partition partial sums [128, 3]; the host combines
them in float64 and forms the final scalar.

Per-row top-32 on device: per-row top-8 of each column block (nc.vector.max)
produces a candidate set; 4 rounds of max+match_replace on the narrow
candidate buffer yield the exact top-32 values provided no block holds >8 of
the row's top-32 elements. kernel() verifies that property on the actual
input on the host (cheap numpy check) and falls back to a smaller block size
or a full-width exact variant if ever violated.
"""

import numpy as np

N = 8192  # attention matrix is [N, N]
D = 1024  # reconstruction feature dim
K = 32  # top-k
ALPHA = 0.1
N_CORES = 8
ROWS = N // N_CORES  # rows per core = 1024
P = 128  # SBUF partitions
NT = ROWS // P  # row-tiles per core = 8

_BUILDS: dict = {}


def _build_bass(blk: int):
    """Build the per-core Bass module.

    blk > 0: level-1 block size for the block-top8 candidate pass.
    blk == 0: exact full-width fallback (4 rounds of max+match_replace over
    the whole 8192-wide row).
    """
    import concourse.tile as tile
    from concourse import bacc, mybir

    f32 = mybir.dt.float32
    Sq = mybir.ActivationFunctionType.Square
    AX = mybir.AxisListType.X
    ADD = mybir.AluOpType.add

    # Bacc (not raw Bass): its compile() pass splits multi-wait sync_infos,
    # which the TRN2 ISA requires (at most one wait per instruction).
    nc = bacc.Bacc()
    attn_in = nc.declare_dram_parameter("attn", [ROWS, N], f32, isOutput=False)
    # host passes x and NEGATED y; an SWDGE accumulate-add DMA computes
    # x + (-y) inline in the SDMA datapath, so no engine does the subtract.
    x_in = nc.declare_dram_parameter("x", [ROWS, D], f32, isOutput=False)
    yneg_in = nc.declare_dram_parameter("yneg", [ROWS, D], f32, isOutput=False)
    out_ext = nc.declare_dram_parameter("out", [P, 4], f32, isOutput=True)

    with tile.TileContext(nc) as tc:
        with (
            tc.tile_pool(name="attn_p", bufs=4) as attn_p,
            tc.tile_pool(name="xy_p", bufs=2) as xy_p,
            tc.tile_pool(name="small_p", bufs=2) as small_p,
            tc.tile_pool(name="acc_p", bufs=1) as acc_p,
        ):
            # acc columns: [0:NT) sum(x-y)^2, [NT:2NT) sum(attn^2) pieces,
            # [2NT:3NT) sum(top32^2) pieces, [3NT:4NT) fallback extras.
            acc = acc_p.tile([P, 4 * NT], f32)
            nc.vector.memset(acc[:], 0.0)

            for t in range(NT):
                a = attn_p.tile([P, N], f32, tag="a")
                nc.sync.dma_start(out=a[:], in_=attn_in[t * P : (t + 1) * P, :])

                top = small_p.tile([P, K], f32, tag="top")
                if blk > 0:
                    nb = N // blk
                    cw = nb * 8
                    cand = small_p.tile([P, cw], f32, tag="cand")
                    for b in range(nb):
                        nc.vector.max(
                            out=cand[:, b * 8 : (b + 1) * 8],
                            in_=a[:, b * blk : (b + 1) * blk],
                        )
                    for r in range(K // 8):
                        nc.vector.max(out=top[:, r * 8 : (r + 1) * 8], in_=cand[:])
                        if r < K // 8 - 1:
                            nc.vector.match_replace(
                                out=cand[:],
                                in_to_replace=top[:, r * 8 : (r + 1) * 8],
                                in_values=cand[:],
                                imm_value=0.0,
                            )
                    # sum(top32^2) for this tile
                    nc.scalar.activation(
                        out=top[:], in_=top[:], func=Sq,
                        accum_out=acc[:, 2 * NT + t : 2 * NT + t + 1],
                    )
                    # sum(attn^2) for this tile (in-place square; `a` is dead after)
                    nc.scalar.activation(
                        out=a[:], in_=a[:], func=Sq,
                        accum_out=acc[:, NT + t : NT + t + 1],
                    )
                else:
                    # Exact fallback: extract top-32 directly from the full row.
                    # match_replace zeroes the extracted values in `a`, so
                    # sum(attn^2) = sum(a_modified^2) + sum(top24_extracted^2).
                    for r in range(K // 8):
                        nc.vector.max(out=top[:, r * 8 : (r + 1) * 8], in_=a[:])
                        if r < K // 8 - 1:
                            nc.vector.match_replace(
                                out=a[:],
                                in_to_replace=top[:, r * 8 : (r + 1) * 8],
                                in_values=a[:],
                                imm_value=0.0,
                            )
                    # first 24 values were zeroed out of `a`
                    nc.scalar.activation(
                        out=top[:, : K - 8], in_=top[:, : K - 8], func=Sq,
                        accum_out=acc[:, 3 * NT + t : 3 * NT + t + 1],
                    )
                    # last 8 remain in `a`
                    nc.scalar.activation(
                        out=top[:, K - 8 :], in_=top[:, K - 8 :], func=Sq,
                        accum_out=acc[:, 2 * NT + t : 2 * NT + t + 1],
                    )
                    nc.scalar.activation(
                        out=a[:], in_=a[:], func=Sq,
                        accum_out=acc[:, NT + t : NT + t + 1],
                    )

                xt = xy_p.tile([P, D], f32, tag="xt")
                nc.sync.dma_start(out=xt[:], in_=x_in[t * P : (t + 1) * P, :])
                nc.gpsimd.dma_start(
                    out=xt[:], in_=yneg_in[t * P : (t + 1) * P, :],
                    accum_op=mybir.AluOpType.add,
                )
                nc.scalar.activation(
                    out=xt[:], in_=xt[:], func=Sq, accum_out=acc[:, t : t + 1]
                )

            osb = acc_p.tile([P, 4], f32)
            nc.vector.tensor_reduce(out=osb[:, 0:1], in_=acc[:, 0:NT], axis=AX, op=ADD)
            nc.vector.tensor_reduce(
                out=osb[:, 1:2], in_=acc[:, NT : 2 * NT], axis=AX, op=ADD
            )
            # col2: sum(top32^2) = tail-8 partials (+ top-24 partials in fallback)
            nc.vector.tensor_reduce(
                out=osb[:, 2:3], in_=acc[:, 2 * NT : 3 * NT], axis=AX, op=ADD
            )
            nc.vector.tensor_reduce(
                out=osb[:, 3:4], in_=acc[:, 3 * NT : 4 * NT], axis=AX, op=ADD
            )
            nc.sync.dma_start(out=out_ext[:], in_=osb[:])

    nc.finalize()  # runs Bacc.compile(): wait splitting + register allocation
    return nc


def _get_nc(blk: int):
    if blk not in _BUILDS:
        _BUILDS[blk] = _build_bass(blk)
    return _BUILDS[blk]


def _pick_blk(attn: np.ndarray) -> int:
    """Choose the largest safe level-1 block size for this input.

    Safe means: for every row, no block contains more than 8 elements that
    are >= the row's 32nd-largest value (so block-top8 candidates provably
    contain every valid top-32 choice).
    """
    t32 = np.partition(attn, N - K, axis=1)[:, N - K]
    ge = attn >= t32[:, None]
    for blk in (256, 128):
        nb = N // blk
        cnt = ge.reshape(N, nb, blk).sum(axis=2, dtype=np.int32)
        if cnt.max() <= 8:
            return blk
    return 0


def _combine(results) -> np.float32:
    S = np.zeros(4, dtype=np.float64)
    for r in results:
        S += r["out"].astype(np.float64).sum(axis=0)
    sxy, sattn, stop_tail, stop_head = S
    # fallback path splits top32^2 across cols 2 and 3 and leaves the
    # head-24 part out of sattn (match_replace zeroed those entries)
    sattn = sattn + stop_head
    stop = stop_tail + stop_head
    loss = sxy / (N * D) + ALPHA * (sattn - stop) / (N * N)
    return np.float32(loss)


def _shard(x: np.ndarray, y: np.ndarray, attn: np.ndarray):
    in_maps = []
    for c in range(N_CORES):
        r0, r1 = c * ROWS, (c + 1) * ROWS
        in_maps.append(
            {
                "attn": np.ascontiguousarray(attn[r0:r1]),
                "x": np.ascontiguousarray(x[r0:r1]),
                "yneg": -y[r0:r1],
            }
        )
    return in_maps


def kernel(x: np.ndarray, y: np.ndarray, attn: np.ndarray) -> np.ndarray:
    from concourse.bass_utils import run_bass_kernel_spmd

    x = np.asarray(x, dtype=np.float32)
    y = np.asarray(y, dtype=np.float32)
    attn = np.asarray(attn, dtype=np.float32)

    nc = _get_nc(_pick_blk(attn))
    res = run_bass_kernel_spmd(nc, _shard(x, y, attn), list(range(N_CORES)))
    return np.asarray(_combine(res.results))
